# revision 7
# baseline (speedup 1.0000x reference)
"""Trainium2 Bass kernel for nn_CA1Replace: 1D cellular automaton (mirrored
rule-110 variant), 32 rows x 16384 cells, 64 iterations, all 65 states out.

Layout (per core, 4 rows):
  Each row is cut into 32 segments of 512 cells. SBUF partition q = s*4 + r
  (segment-major) holds cells [s*512 - HALO, s*512 + 512 + HALO) of row r.
  With wrap=False every partition can run all 64 iterations locally: the
  halo is wide enough (HALO=66 > 64+1) that boundary corruption never
  reaches the core 512 cells. The one true boundary condition that needs
  enforcement is the virtual cell at index 16384 (it must stay 0 every
  iteration); that is a [4 partitions x 1] memset per iteration.

Update rule: new = lookup[L + 2C + 4R], lookup = bits of 110
           = (L+C+R <= 2) && (L+C >= 1)   (verified over all 8 cases)
computed as:  u = L + C ; v = u + R ; new = (v is_le 2) logical_and u
"""

import numpy as np

import concourse.bass as bass
import concourse.mybir as mybir
from concourse.tile import TileContext
from concourse.vector_clock import ScopedClock
from concourse.bass_utils import run_bass_kernel_spmd

B = 32          # batch rows
W = 16384       # row width
ITERS = 64
NT = ITERS + 1  # states including t=0
NCORES = 8
RPC = B // NCORES     # rows per core = 4
SEG = 512             # core cells per partition
NSEG = W // SEG       # 32 segments
HALO = 66
TW = SEG + 2 * HALO   # 644 tile width
MASK_COL = W - (NSEG - 1) * SEG + HALO  # local col of cell 16384 in last segs = 578

_f32 = mybir.dt.float32
_bf16 = mybir.dt.bfloat16
AO = mybir.AluOpType

# PERM[k] = segment index stored at partition block k (block = 4 partitions,
# one per row). Segment 31 (owner of virtual cell 16384, which must be
# re-zeroed every iteration) is placed at block 24 so the memset's partition
# range starts at 96 — engine ops require a 32-aligned base partition.
PERM = list(range(NSEG))
PERM[24], PERM[31] = 31, 24
MASK_PART = 24 * RPC  # 96


def _patch_tile_drain():
    """walrus (this version) allows only 1 sync-wait on a CTRL instruction;
    Tile's kernel-tail drain accumulates one wait per used processor.
    Split the extra waits onto dedicated nops."""
    if getattr(TileContext, "_drain_patched", False):
        return

    def _drain_and_barrier(self, tick_clock, wait_clock):
        nc = self.nc
        drain_inst = nc.sync.drain()
        wait_clock.add_sem_waits(
            drain_inst.ins, ScopedClock({None: tick_clock.global_clock})
        )
        si = drain_inst.ins.sync_info
        waits = list(si.on_wait or [])
        upd = list(si.on_update or [])
        if len(waits) > 1:
            drain_inst.ins.sync_info = mybir.SyncInfo(on_wait=waits[:1], on_update=upd)
            for w in waits[1:]:
                nop_inst = nc.sync.nop()
                nop_inst.ins.sync_info = mybir.SyncInfo(on_wait=[w], on_update=[])
        nc.all_engine_barrier()
        assert self.sems is not None
        popped = nc._tile_sem_poison_stack.pop()
        assert popped is self._sem_poison
        nc.clear_and_free_semaphores(list(self.sems.allocated().values()))
        nc.all_engine_barrier()

    TileContext._drain_and_barrier = _drain_and_barrier
    TileContext._drain_patched = True


def _legalize_sync_waits(nc):
    """This walrus build accepts at most ONE sync-wait per instruction, but
    Tile attaches as many as the dependence structure needs. Hoist extras
    onto fresh same-engine nops inserted directly before the offender (the
    engine is in-order, so serializing the waits is equivalent)."""
    for f in nc.m.functions:
        for bb in f.blocks:
            insts = list(bb.instructions)
            new_list = []
            changed = False
            for ins in insts:
                si = ins.sync_info
                if si is not None and si.on_wait and len(si.on_wait) > 1:
                    changed = True
                    waits = list(si.on_wait)
                    eng = ins.engine
                    for w in waits[:-1]:
                        h = nc.engines[eng].nop()
                        nop_ins = h.ins
                        nop_ins.sync_info = mybir.SyncInfo(on_wait=[w], on_update=[])
                        new_list.append(nop_ins)
                    ins.sync_info = mybir.SyncInfo(
                        on_wait=[waits[-1]], on_update=list(si.on_update or [])
                    )
                new_list.append(ins)
            if changed:
                # the nop() calls above appended into the current bb; strip
                # them from wherever they landed, then install our order
                appended = {id(x) for x in new_list} - {id(x) for x in insts}
                for f2 in nc.m.functions:
                    for bb2 in f2.blocks:
                        cur = list(bb2.instructions)
                        stripped = [
                            x
                            for x in cur
                            if not (id(x) in appended and bb2 is not bb)
                        ]
                        if bb2 is bb:
                            bb2.instructions = new_list
                        elif len(stripped) != len(cur):
                            bb2.instructions = stripped


def _build():
    _patch_tile_drain()
    nc = bass.Bass("TRN2", target_bir_lowering=False, debug=False)
    x = nc.dram_tensor("xp", [128, TW], _f32, kind="ExternalInput")
    out = nc.dram_tensor("out", [128, NT * TW], _bf16, kind="ExternalOutput")

    with TileContext(nc) as tc:
        with (
            tc.tile_pool(name="xin", bufs=1) as xpool,
            tc.tile_pool(name="hist", bufs=1) as hpool,
            tc.tile_pool(name="scratch", bufs=2) as spool,
        ):
            xt = xpool.tile([128, TW], _f32)
            nc.sync.dma_start(xt[:, :], x[:, :])

            hist = hpool.tile([128, NT * TW], _bf16)
            hv = hist[:, :].rearrange("p (t w) -> p t w", w=TW)
            # pin guard columns 0 and TW-1 of every state tile to 0
            nc.vector.memset(hv[:, :, 0], 0.0)
            nc.vector.memset(hv[:, :, TW - 1], 0.0)

            # t=0 state: threshold
            nc.vector.tensor_scalar(
                hist[:, 0:TW], xt[:, :], 0.5, None, AO.is_ge
            )

            dma_lo = 0
            for t in range(1, NT):
                sp = hist[:, (t - 1) * TW : t * TW]
                st = hist[:, t * TW : (t + 1) * TW]
                u = spool.tile([128, TW], _bf16, tag="u")
                v = spool.tile([128, TW], _bf16, tag="v")
                nc.vector.tensor_tensor(
                    u[:, 1 : TW - 1], sp[:, 0 : TW - 2], sp[:, 1 : TW - 1], AO.add
                )
                nc.vector.tensor_tensor(
                    v[:, 1 : TW - 1], u[:, 1 : TW - 1], sp[:, 2:TW], AO.add
                )
                nc.vector.scalar_tensor_tensor(
                    st[:, 1 : TW - 1],
                    v[:, 1 : TW - 1],
                    2.0,
                    u[:, 1 : TW - 1],
                    AO.is_le,
                    AO.logical_and,
                )
                # virtual cell W must stay 0 (it lives in the last-segment
                # partitions only)
                nc.vector.memset(
                    st[MASK_PART : MASK_PART + RPC, MASK_COL : MASK_COL + 1], 0.0
                )

                if t % 8 == 0 or t == NT - 1:
                    nc.sync.dma_start(
                        out[:, dma_lo * TW : (t + 1) * TW],
                        hist[:, dma_lo * TW : (t + 1) * TW],
                    )
                    dma_lo = t + 1
    _legalize_sync_waits(nc)
    return nc


_nc_cache = None


def _get_nc():
    global _nc_cache
    if _nc_cache is None:
        _nc_cache = _build()
    return _nc_cache


def _prep_core(xc: np.ndarray) -> np.ndarray:
    """xc [RPC, W] float32 -> [128, TW] device layout with halos."""
    pad = np.zeros((RPC, W + 2 * HALO), np.float32)
    pad[:, HALO : HALO + W] = xc
    xp = np.empty((128, TW), np.float32)
    for k in range(NSEG):
        s = PERM[k]
        xp[k * RPC : (k + 1) * RPC, :] = pad[:, s * SEG : s * SEG + TW]
    return xp


def _post_core(o: np.ndarray) -> np.ndarray:
    """o [128, NT*TW] bf16 -> [RPC, NT, W] int32."""
    a = np.asarray(o).astype(np.float32).reshape(128, NT, TW)[:, :, HALO : HALO + SEG]
    a = a.reshape(NSEG, RPC, NT, SEG)  # [block k, r, t, cell]
    inv = np.argsort(PERM)  # inv[s] = block holding segment s
    a = a[inv]  # now indexed by segment s in order
    a = a.transpose(1, 2, 0, 3).reshape(RPC, NT, W)
    return a.astype(np.int32)


def run_cores(x: np.ndarray, trace: bool = False):
    nc = _get_nc()
    in_maps = [
        {"xp": _prep_core(x[RPC * c : RPC * (c + 1)].astype(np.float32))}
        for c in range(NCORES)
    ]
    res = run_bass_kernel_spmd(nc, in_maps, list(range(NCORES)), trace=trace)
    return res


def kernel(x: np.ndarray, lookup: np.ndarray) -> np.ndarray:
    # the boolean form hardwired in the device kernel implements exactly
    # this lookup table (rule 110, low-bit-first)
    assert np.array_equal(np.asarray(lookup).ravel(), [0, 1, 1, 1, 0, 1, 1, 0])
    res = run_cores(np.asarray(x))
    out = np.stack([_post_core(r["out"]) for r in res.results])  # [8, RPC, NT, W]
    return out.reshape(B, NT, W).astype(np.int32)


# revision 8
# speedup vs baseline: 1.0534x; 1.0534x over previous
"""Trainium2 Bass kernel for nn_CA1Replace: 1D cellular automaton
(rule 110, low-bit-first lookup => mirrored rule), 32 rows x 16384 cells,
64 iterations, all 65 states returned as [32, 65, 16384] int32.

Sharding: pure data parallelism — 4 rows per NeuronCore across 8 cores.

Per-core algorithm (PE + ACT + DVE pipeline):
  Layout: state s_t is [128, 512] fp8_e4m3 in SBUF; partition p =
  cell-within-segment, column = r*128 + g (row r in 0..3, segment g in
  0..127), cell index w = g*128 + p.

  The update rule new = lookup[L + 2C + 4R] with lookup = bits of 110
  collapses to an interval test of one linear form (verified over all 8
  neighborhoods):   v' = 2L + 2C + R,   new = [2 <= v' <= 4].

  Per iteration (split into CHAINS independent column groups so the three
  engines pipeline):
    PE : v' = banded matmul (weights 2/2/1 on the sub/main/super diagonal)
         + 2 narrow boundary matmuls carrying the cross-segment neighbor
         terms (out partition strips 0:32 and 96:128)      -> PSUM fp32
    ACT: rel = Relu(4.5 - v')   (rel > 0  <=>  v' <= 4)    -> SBUF bf16
    DVE: s' = (v' >= 2) logical_and rel                    -> SBUF fp8

  Boundary conditions (wrap=False) fall out of the matmul structure: the
  first/last column of each row receives no left/right boundary term.

  All 65 states accumulate in one big SBUF history buffer and are DMA'd
  out as fp8 bytes in chunks; the host decodes bytes -> {0,1} and
  un-transposes the layout.
"""

import numpy as np
import ml_dtypes

import concourse.bass as bass
import concourse.mybir as mybir
from concourse.tile import TileContext
from concourse.vector_clock import ScopedClock
from concourse.bass_utils import run_bass_kernel_spmd

B, W, ITERS, NCORES = 32, 16384, 64, 8
NT = ITERS + 1
RPC = B // NCORES          # 4 rows per core
NCOL = RPC * 128           # 512 state columns
NSEG = W // 128            # 128 segments per row

_f32 = mybir.dt.float32
_bf16 = mybir.dt.bfloat16
_fp8 = mybir.dt.float8e4
_f8np = ml_dtypes.float8_e4m3
AO = mybir.AluOpType
AF = mybir.ActivationFunctionType

DMA_CHUNK = 8   # state tiles per output DMA
CHAINS = 4      # independent pipeline chains (128 cols each)
PSUM_BUFS = 8
REL_BUFS = 8


def _patch_tile_drain():
    """This walrus build accepts at most ONE sync-wait per CTRL
    instruction; Tile's kernel-tail drain accumulates one wait per used
    processor. Split the extra waits onto dedicated nops."""
    if getattr(TileContext, "_drain_patched", False):
        return

    def _drain_and_barrier(self, tick_clock, wait_clock):
        nc = self.nc
        drain_inst = nc.sync.drain()
        wait_clock.add_sem_waits(
            drain_inst.ins, ScopedClock({None: tick_clock.global_clock})
        )
        si = drain_inst.ins.sync_info
        waits = list(si.on_wait or [])
        upd = list(si.on_update or [])
        if len(waits) > 1:
            drain_inst.ins.sync_info = mybir.SyncInfo(on_wait=waits[:1], on_update=upd)
            for w in waits[1:]:
                nop_inst = nc.sync.nop()
                nop_inst.ins.sync_info = mybir.SyncInfo(on_wait=[w], on_update=[])
        nc.all_engine_barrier()
        assert self.sems is not None
        popped = nc._tile_sem_poison_stack.pop()
        assert popped is self._sem_poison
        nc.clear_and_free_semaphores(list(self.sems.allocated().values()))
        nc.all_engine_barrier()

    TileContext._drain_and_barrier = _drain_and_barrier
    TileContext._drain_patched = True


def _legalize_sync_waits(nc):
    """Hoist extra sync-waits (walrus allows one per instruction) onto
    fresh same-engine nops inserted directly before the offender; the
    engine is in-order so serializing the waits is equivalent."""
    for f in nc.m.functions:
        for bb in f.blocks:
            insts = list(bb.instructions)
            new_list = []
            changed = False
            for ins in insts:
                si = ins.sync_info
                if si is not None and si.on_wait and len(si.on_wait) > 1:
                    changed = True
                    waits = list(si.on_wait)
                    eng = ins.engine
                    for w in waits[:-1]:
                        h = nc.engines[eng].nop()
                        nop_ins = h.ins
                        nop_ins.sync_info = mybir.SyncInfo(on_wait=[w], on_update=[])
                        new_list.append(nop_ins)
                    ins.sync_info = mybir.SyncInfo(
                        on_wait=[waits[-1]], on_update=list(si.on_update or [])
                    )
                new_list.append(ins)
            if changed:
                appended = {id(x) for x in new_list} - {id(x) for x in insts}
                for f2 in nc.m.functions:
                    for bb2 in f2.blocks:
                        cur = list(bb2.instructions)
                        stripped = [
                            x for x in cur if not (id(x) in appended and bb2 is not bb)
                        ]
                        if bb2 is bb:
                            bb2.instructions = new_list
                        elif len(stripped) != len(cur):
                            bb2.instructions = stripped


def _build():
    _patch_tile_drain()
    nc = bass.Bass("TRN2", target_bir_lowering=False, debug=False)
    x = nc.dram_tensor("xp", [128, NCOL], _f32, kind="ExternalInput")
    wts = nc.dram_tensor("wts", [128, 512], _fp8, kind="ExternalInput")
    out = nc.dram_tensor("out", [128, NT * NCOL], _fp8, kind="ExternalOutput")

    with TileContext(nc) as tc:
        with (
            tc.tile_pool(name="cst", bufs=1) as cst,
            tc.tile_pool(name="hist", bufs=1) as hp,
            tc.tile_pool(name="ps", bufs=PSUM_BUFS, space="PSUM") as ps,
            tc.tile_pool(name="rel", bufs=REL_BUFS) as rp,
        ):
            wt = cst.tile([128, 512], _fp8, tag="wts")
            nc.sync.dma_start(wt[:, :], wts[:, :])
            WM = wt[:, 0:128]
            BL = wt[:, 128:160]  # [128, 32], only [127, 0] = 2 (L of cell 0)
            BR = wt[:, 160:192]  # [128, 32], only [0, 31] = 1 (R of cell 127)
            bias = cst.tile([128, 1], _f32, tag="bias")
            nc.vector.memset(bias[:, :], 4.5)

            xt = cst.tile([128, NCOL], _f32, tag="xin")
            nc.sync.dma_start(xt[:, :], x[:, :])

            hist = hp.tile([128, NT * NCOL], _fp8)
            # s_0 = threshold(x)
            nc.vector.tensor_scalar(hist[:, 0:NCOL], xt[:, :], 0.5, None, AO.is_ge)

            dma_lo = 0
            CW = NCOL // CHAINS
            for t in range(1, NT):
                sp = hist[:, (t - 1) * NCOL : t * NCOL]
                st = hist[:, t * NCOL : (t + 1) * NCOL]
                vts, mrs = [], []
                for S in range(CHAINS):
                    mv = sp[:, S * CW : (S + 1) * CW]
                    mrs.append(mv.rearrange("p (r g) -> p r g", g=128))
                    vtile = ps.tile([128, CW], _f32, tag="v")
                    vts.append(vtile)
                # grouped by stationary so repeated LDWEIGHTS are cheap
                for S in range(CHAINS):
                    nc.tensor.matmul(
                        vts[S][:, :],
                        WM,
                        sp[:, S * CW : (S + 1) * CW],
                        start=True,
                        stop=False,
                    )
                for S in range(CHAINS):
                    vr = vts[S][:, :].rearrange("p (r g) -> p r g", g=128)
                    nc.tensor.matmul(
                        vr[0:32, :, 1:128],
                        BL,
                        mrs[S][:, :, 0:127],
                        start=False,
                        stop=False,
                    )
                for S in range(CHAINS):
                    vr = vts[S][:, :].rearrange("p (r g) -> p r g", g=128)
                    nc.tensor.matmul(
                        vr[96:128, :, 0:127],
                        BR,
                        mrs[S][:, :, 1:128],
                        start=False,
                        stop=True,
                        tile_position=(0, 96),
                    )
                for S in range(CHAINS):
                    rel = rp.tile([128, CW], _bf16, tag="rel")
                    nc.scalar.activation(
                        rel[:, :], vts[S][:, :], AF.Relu, bias=bias[:, :], scale=-1.0
                    )
                    nc.vector.scalar_tensor_tensor(
                        st[:, S * CW : (S + 1) * CW],
                        vts[S][:, :],
                        2.0,
                        rel[:, :],
                        AO.is_ge,
                        AO.logical_and,
                    )
                if t % DMA_CHUNK == 0 or t == NT - 1:
                    nc.sync.dma_start(
                        out[:, dma_lo * NCOL : (t + 1) * NCOL],
                        hist[:, dma_lo * NCOL : (t + 1) * NCOL],
                    )
                    dma_lo = t + 1
    _legalize_sync_waits(nc)
    return nc


_nc_cache = None


def _get_nc():
    global _nc_cache
    if _nc_cache is None:
        _nc_cache = _build()
    return _nc_cache


def _weights_np() -> np.ndarray:
    # v' = 2L + 2C + R; stationary[k, m] = weight of cell k into output m
    w = np.zeros((128, 512), np.float32)
    WMn = w[:, 0:128]
    for m in range(128):
        if m - 1 >= 0:
            WMn[m - 1, m] = 2.0  # L
        WMn[m, m] = 2.0  # C
        if m + 1 < 128:
            WMn[m + 1, m] = 1.0  # R
    w[127, 128 + 0] = 2.0  # BL narrow: L of the first cell of a segment
    w[0, 160 + 31] = 1.0  # BR narrow: R of the last cell of a segment
    return w.astype(_f8np)


def _prep_core(xc: np.ndarray) -> np.ndarray:
    # x_pre[p, r*128 + g] = x[r, g*128 + p]
    return (
        xc.reshape(RPC, NSEG, 128).transpose(2, 0, 1).reshape(128, NCOL)
    ).astype(np.float32)


def _post_core(o: np.ndarray) -> np.ndarray:
    raw = np.asarray(o)
    if raw.dtype != np.uint8:
        raw = raw.view(np.uint8)
    bits = (raw != 0).astype(np.int32)  # fp8 0.0 == 0x00, 1.0 == 0x38
    a = bits.reshape(128, NT, RPC, NSEG)  # [p, t, r, g]
    return a.transpose(2, 1, 3, 0).reshape(RPC, NT, W)


def run_cores(x: np.ndarray, trace: bool = False):
    nc = _get_nc()
    wn = _weights_np()
    in_maps = [
        {
            "xp": _prep_core(np.asarray(x)[RPC * c : RPC * (c + 1)]),
            "wts": wn,
        }
        for c in range(NCORES)
    ]
    return run_bass_kernel_spmd(nc, in_maps, list(range(NCORES)), trace=trace)


def kernel(x: np.ndarray, lookup: np.ndarray) -> np.ndarray:
    # the interval form hardwired in the device kernel implements exactly
    # this lookup table (rule 110, low-bit-first)
    assert np.array_equal(np.asarray(lookup).ravel(), [0, 1, 1, 1, 0, 1, 1, 0])
    res = run_cores(np.asarray(x))
    out = np.stack([_post_core(r["out"]) for r in res.results])
    return out.reshape(B, NT, W).astype(np.int32)
